# revision 1
# baseline (speedup 1.0000x reference)
"""Trainium2 Bass kernel for leave-one-out Nadaraya-Watson regression
(nn_Net_72877005078649) — fast-Gauss-transform formulation, v2.

Per output channel o this is 1D Gaussian kernel regression; the kernel
factorizes through a G=12 grid (a = b = h/sqrt(2), trapezoid aliasing
~1e-4):  K_h(x,z) ~= kappa * sum_g exp(-(c_g-x)^2/h^2) exp(-(z-c_g)^2/h^2)

v2 design notes (instruction-count-bound on TRN2):
 - host ships transposed/padded layouts: xT/tXT [128(d-pad), n] f32r,
   W1T [128,128], W2rep [128,128] where W2rep[:,p] = W2[p%10,:] — one
   K=128 matmul per 512-col chunk yields XwRep[(g,o)-partition, n]
   directly (no per-tile W2, no transposes, no broadcast ops).
 - source tables in transposed orientation: den[g,o] = sum_n E comes
   free via ACT accum_out on the exp; num via one DVE STT(mult Yrep)
   with accum_out per chunk-pair; YrepT [128, N] = Y[n, p%10] bf16 is
   shipped from host.
 - train side processed as 4 pairs of 512-col chunks ([128,1024] ops).
 - diagonal: train_X == x by construction (the reference's LOO eye-mask
   requires it), so K_ii == 1 exactly: out = (num - Y_d)/(den - 1).
 - query side: Eq[(g,o),b] from the same W2rep path; num/den via one
   K=128 matmul with kappa-and-diagonal-masked tables AA [128,20].

Sharding: queries split across 8 cores (512/core); train replicated.
"""

import numpy as np

N = 4096
D = 64
HID = 128
O = 10
NCORES = 8
BQ = N // NCORES
G = 12
GO = G * O
GRID_LO = -6.5
GRID_HI = 6.5
NPAIR = 4               # train chunk pairs, 1024 cols each

_cache = {}


def _host_consts(h: float):
    c = np.linspace(GRID_LO, GRID_HI, G).astype(np.float32)
    delta = float(c[1] - c[0])
    kappa = 2.0 * delta / (np.sqrt(2.0 * np.pi) * h)
    # consts[128, 22] = cq[128,1] | kmask2[128,20] | -cq[128,1]
    consts = np.zeros((128, 22), np.float32)
    for p in range(128):
        consts[p, 0] = c[min(p // O, G - 1)]
        consts[p, 21] = -c[min(p // O, G - 1)]
    for p in range(120):
        consts[p, 1 + p % O] = kappa          # num mask
        consts[p, 11 + p % O] = kappa         # den mask
    return consts, kappa


def _host_tensors(x, train_X, Y, W1, W2):
    # transposed, d-padded to 128 partitions
    import jax.numpy as jnp
    tXT = np.asarray(jnp.asarray(train_X.T, dtype=jnp.bfloat16))
    W1T = np.asarray(jnp.asarray(W1.T, dtype=jnp.bfloat16))
    # W2rep[hid, p] = W2[p%10, hid]
    W2rep = np.empty((HID, 128), np.float32)
    for p in range(128):
        W2rep[:, p] = W2[p % O, :]
    # YrepT[p, n] = Y[n, p%10], bf16
    Yrep = np.asarray(Y[:, [p % O for p in range(128)]].T)  # [128, N] f32
    Yrep16 = np.asarray(jnp.asarray(Yrep, dtype=jnp.bfloat16))
    return tXT, W1T, W2rep, Yrep16


def _build(h: float):
    import concourse.bass as bass
    import concourse.bacc as bacc
    import concourse.tile as tile
    from concourse import mybir
    from concourse.masks import make_identity

    f32 = mybir.dt.float32
    f32r = mybir.dt.float32r
    bf16 = mybir.dt.bfloat16
    AF = mybir.ActivationFunctionType
    ALU = mybir.AluOpType

    s_n = 1.0 / (h * h)

    nc = bacc.Bacc("TRN2", target_bir_lowering=False, debug=False, num_devices=1)
    xqT = nc.dram_tensor("xqT", [64, BQ], bf16, kind="ExternalInput").ap()
    tXT = nc.dram_tensor("tXT", [64, N], bf16, kind="ExternalInput").ap()
    W1Td = nc.dram_tensor("W1Td", [64, HID], bf16, kind="ExternalInput").ap()
    W2rd = nc.dram_tensor("W2rd", [HID, 128], f32r, kind="ExternalInput").ap()
    Yrd = nc.dram_tensor("Yrd", [128, N], bf16, kind="ExternalInput").ap()
    ydd = nc.dram_tensor("ydd", [BQ, O], f32, kind="ExternalInput").ap()
    constsd = nc.dram_tensor("constsd", [128, 22], f32, kind="ExternalInput").ap()
    out = nc.dram_tensor("out", [BQ, O], f32, kind="ExternalOutput").ap()

    with tile.TileContext(nc) as tc:
        with (
            tc.tile_pool(name="S", bufs=1) as S,
            tc.tile_pool(name="W", bufs=2) as W,
            tc.tile_pool(name="PS", bufs=1, space="PSUM") as PS,
        ):
            # ---- ACT warmup (exp table) ----
            warm = S.tile([1, 16], f32)
            nc.vector.memset(warm, 0.0)
            nc.scalar.activation(out=warm, in_=warm, func=AF.Exp)

            ident = S.tile([128, 128], f32)
            make_identity(nc, ident)

            # ---- input DMAs: strict priority on the sync queue ----
            w1T = S.tile([64, HID], bf16)
            nc.sync.dma_start(out=w1T, in_=W1Td)
            xq_sb = S.tile([64, BQ], bf16)
            nc.sync.dma_start(out=xq_sb, in_=xqT)
            consts = S.tile([128, 22], f32)
            nc.sync.dma_start(out=consts, in_=constsd)
            w2r = S.tile([HID, 128], f32r)
            nc.sync.dma_start(out=w2r, in_=W2rd)
            ydt = S.tile([128, 4 * O], f32)
            nc.sync.dma_start(
                out=ydt.rearrange("p (j o) -> p j o", o=O),
                in_=ydd.rearrange("(j p) o -> p j o", p=128))
            tX_sb = S.tile([64, N], bf16)
            for sl in range(4):
                nc.sync.dma_start(out=tX_sb[:, sl * 1024:(sl + 1) * 1024],
                                  in_=tXT[:, sl * 1024:(sl + 1) * 1024])
            # Yr on the scalar queue, halves, issued after the warmup so the
            # transfers trail the critical smalls on the fabric
            Yr_sb = S.tile([128, N], bf16)
            nc.scalar.dma_start(out=Yr_sb[:, 0:2048], in_=Yrd[:, 0:2048])
            nc.scalar.dma_start(out=Yr_sb[:, 2048:4096], in_=Yrd[:, 2048:4096])

            cq = consts[:, 0:1]
            kmask2 = consts[:, 1:21]
            ncq = consts[:, 21:22]

            nparts = S.tile([128, NPAIR], f32)
            dparts = S.tile([128, NPAIR], f32)
            parts = S.tile([128, 2], f32)
            AA = S.tile([128, 2 * O], f32r)
            Eq = S.tile([128, BQ], f32r)
            dq = S.tile([128, BQ], f32)
            dq2 = S.tile([128, BQ], f32)
            qsb = S.tile([20, BQ], f32)
            nsb = S.tile([128, 4 * O], f32)
            dsb = S.tile([128, 4 * O], f32)
            rsb = S.tile([128, 4 * O], f32)
            osb = S.tile([128, 4 * O], f32)

            # ---- query chunk first (tail only needs Eq + AA) ----
            hpsq = PS.tile([128, 1024], f32, tag="hps", bufs=2, name="hpsq")
            nc.tensor.matmul(hpsq[:, 0:BQ], lhsT=w1T, rhs=xq_sb,
                             start=True, stop=True)
            h1q = W.tile([128, BQ], f32r, tag="h1q", bufs=1)
            nc.scalar.activation(out=h1q, in_=hpsq[:, 0:BQ], func=AF.Relu)
            xrq = PS.tile([128, 1024], f32, tag="xr", bufs=2, name="xrq")
            nc.tensor.matmul(xrq[:, 0:BQ], lhsT=w2r, rhs=h1q,
                             start=True, stop=True)
            nc.vector.tensor_scalar(out=dq, in0=xrq[:, 0:BQ], scalar1=cq,
                                    scalar2=None, op0=ALU.subtract)
            nc.gpsimd.tensor_tensor(out=dq2, in0=dq, in1=dq, op=ALU.mult)
            nc.scalar.activation(out=Eq, in_=dq2, func=AF.Exp, scale=-s_n)

            # ---- 4 train pairs, software-pipelined on PE ----
            hps_t = []
            xr_t = []

            def w1_pair(p):
                n0 = p * 1024
                hps = PS.tile([128, 1024], f32, tag="hps", bufs=2,
                              name=f"hps{p}")
                hps_t.append(hps)
                nc.tensor.matmul(hps[:, 0:512], lhsT=w1T,
                                 rhs=tX_sb[:, n0:n0 + 512],
                                 start=True, stop=True)
                nc.tensor.matmul(hps[:, 512:1024], lhsT=w1T,
                                 rhs=tX_sb[:, n0 + 512:n0 + 1024],
                                 start=True, stop=True)

            w1_pair(0)
            for p in range(NPAIR):
                n0 = p * 1024
                h1 = W.tile([128, 1024], f32r, tag="h1")
                nc.scalar.activation(out=h1, in_=hps_t[p], func=AF.Relu)
                if p + 1 < NPAIR:
                    w1_pair(p + 1)
                xr = PS.tile([128, 1024], f32, tag="xr", bufs=2, name="xr")
                nc.tensor.matmul(xr[:, 0:512], lhsT=w2r, rhs=h1[:, 0:512],
                                 start=True, stop=True)
                nc.tensor.matmul(xr[:, 512:1024], lhsT=w2r, rhs=h1[:, 512:1024],
                                 start=True, stop=True)
                d2 = W.tile([128, 1024], f32, tag="d2")
                if p == NPAIR - 1:
                    nc.scalar.activation(out=d2, in_=xr, func=AF.Square,
                                         bias=ncq, scale=1.0)
                else:
                    db = W.tile([128, 1024], f32, tag="db")
                    nc.vector.tensor_scalar(out=db, in0=xr, scalar1=cq,
                                            scalar2=None, op0=ALU.subtract)
                    nc.gpsimd.tensor_tensor(out=d2, in0=db, in1=db, op=ALU.mult)
                ET = W.tile([128, 1024], bf16, tag="ET")
                nc.scalar.activation(out=ET, in_=d2, func=AF.Exp, scale=-s_n,
                                     accum_out=dparts[:, p:p + 1])
                scr = W.tile([128, 1024], bf16, tag="scr")
                nc.vector.scalar_tensor_tensor(
                    out=scr, in0=ET, scalar=1.0, in1=Yr_sb[:, n0:n0 + 1024],
                    op0=ALU.bypass, op1=ALU.mult,
                    accum_out=nparts[:, p:p + 1])

            # ---- tables -> AA ----
            nc.vector.tensor_reduce(out=parts[:, 0:1], in_=nparts,
                                    axis=mybir.AxisListType.X, op=ALU.add)
            nc.vector.tensor_reduce(out=parts[:, 1:2], in_=dparts,
                                    axis=mybir.AxisListType.X, op=ALU.add)
            PP = parts.ap[0][0]
            parts_b = bass.AP(tensor=parts.tensor, offset=parts.offset,
                              ap=[[PP, 128], [1, 2], [0, O]])
            nc.vector.tensor_tensor(out=AA.rearrange("p (k e) -> p k e", e=O),
                                    in0=parts_b,
                                    in1=kmask2.rearrange("p (k e) -> p k e", e=O),
                                    op=ALU.mult)

            # ---- query contraction + finalize ----
            qps_t = PS.tile([128, 1024], f32, tag="xr", bufs=2, name="qps_t")
            qps = qps_t[0:20, 0:BQ]
            nc.tensor.matmul(qps, lhsT=AA, rhs=Eq, start=True, stop=True)
            nc.scalar.activation(out=qsb, in_=qps, func=AF.Copy)
            fin_t = PS.tile([128, 1024], f32, tag="xr", bufs=2, name="fin_t")
            fin = fin_t[:, 0:128]
            for j in range(4):
                nc.tensor.matmul(
                    fin[0:128, j * 20:(j + 1) * 20],
                    lhsT=qsb[:, j * 128:(j + 1) * 128],
                    rhs=ident[0:20, 0:20],
                    is_transpose=True, start=True, stop=True)

            FP = fin.ap[0][0]
            num4 = bass.AP(tensor=fin.tensor, offset=fin.offset,
                           ap=[[FP, 128], [20, 4], [1, O]])
            den4 = bass.AP(tensor=fin.tensor, offset=fin.offset + O,
                           ap=[[FP, 128], [20, 4], [1, O]])
            nc.vector.tensor_tensor(out=nsb, in0=num4, in1=ydt, op=ALU.subtract)
            nc.vector.tensor_scalar(out=dsb, in0=den4, scalar1=-1.0,
                                    scalar2=None, op0=ALU.add)
            nc.vector.reciprocal(rsb, dsb)
            nc.vector.tensor_tensor(out=osb, in0=nsb, in1=rsb, op=ALU.mult)
            nc.sync.dma_start(
                out=out.rearrange("(j p) o -> p j o", p=128),
                in_=osb.rearrange("p (j o) -> p j o", o=O))

    nc.compile()
    return nc


def build_in_maps(x, train_X, Y, W1, W2, h):
    consts, _ = _host_consts(float(h))
    x = np.ascontiguousarray(x, dtype=np.float32)
    train_X = np.ascontiguousarray(train_X, dtype=np.float32)
    Y = np.ascontiguousarray(Y, dtype=np.float32)
    W1 = np.ascontiguousarray(W1, dtype=np.float32)
    W2 = np.ascontiguousarray(W2, dtype=np.float32)
    tXT, W1T, W2rep, Yrep16 = _host_tensors(x, train_X, Y, W1, W2)
    in_maps = []
    for c in range(NCORES):
        sl = slice(c * BQ, (c + 1) * BQ)
        import jax.numpy as jnp
        xqT = np.asarray(jnp.asarray(x[sl].T, dtype=jnp.bfloat16))
        in_maps.append({
            "xqT": xqT, "tXT": tXT, "W1Td": W1T, "W2rd": W2rep,
            "Yrd": Yrep16, "ydd": Y[sl], "constsd": consts,
        })
    return in_maps


def kernel(x, train_X, Y, W1, W2, h):
    import concourse.bass_utils as bass_utils

    hval = float(h)
    key = ("fgt2", hval)
    if key not in _cache:
        _cache[key] = _build(hval)
    nc = _cache[key]

    in_maps = build_in_maps(x, train_X, Y, W1, W2, h)
    res = bass_utils.run_bass_kernel_spmd(nc, in_maps, core_ids=list(range(NCORES)))
    return np.concatenate([res.results[c]["out"] for c in range(NCORES)], axis=0)



# revision 4
# speedup vs baseline: 1.2791x; 1.2791x over previous
"""Trainium2 Bass kernel for leave-one-out Nadaraya-Watson regression
(nn_Net_72877005078649) — fast-Gauss-transform formulation, v3.

Per output channel o this is 1D Gaussian kernel regression; the kernel
factorizes through a G=12 grid (a = b = h/sqrt(2), trapezoid aliasing
~1e-4):  K_h(x,z) ~= kappa * sum_g exp(-(c_g-x)^2/h^2) exp(-(z-c_g)^2/h^2)

v3 design notes (vs v2 baseline at ~41.5us):
 - per-core input roll: core c's train data is rotated so its own 512
   queries are train chunk 0 — the query-side Eq is just cols 0:512 of
   pair-0's train-side exp table ET0. The whole separate query
   projection chain (W1/relu/W2/sub/sq/exp on [*,512]) is gone.
 - inputs ship packed on 128 partitions, pair-contiguous: tX_q
   [128,512] bf16 holds two 512-col train chunks stacked on partition
   halves (W1T duplicated on rows 0:64 and 64:128 of wpack so both
   halves matmul with base-partition-aligned lhsT). Few large
   contiguous DMAs across 4 engine queues instead of many tiny-packet
   strided ones.
 - finalize: fin_j [128q,20] = matmul(lhsT=ET0[:, j*128:+128] bf16,
   rhs=AA bf16) gives num|den directly in query-partition layout — no
   identity transposes, no PSUM->SBUF copy, no make_identity.
 - elementwise rebalance per pair across ACT/DVE/GpSimd; ACT exp keeps
   den via accum_out, num via STT (DVE on odd / GpSimd on even pairs).
 - diagonal: train_X == x by construction, K_ii == 1 exactly:
   out = (num - Y_d)/(den - 1).

Sharding: queries split across 8 cores (512/core); train replicated.
"""

import numpy as np

N = 4096
D = 64
HID = 128
O = 10
NCORES = 8
BQ = N // NCORES
G = 12
GRID_LO = -6.5
GRID_HI = 6.5
NPAIR = 4               # train chunk pairs, 1024 cols each

_cache = {}


def _host_consts(h: float):
    c = np.linspace(GRID_LO, GRID_HI, G).astype(np.float32)
    delta = float(c[1] - c[0])
    kappa = 2.0 * delta / (np.sqrt(2.0 * np.pi) * h)
    # cbase[128, 22] = cq[128,1] | kmask2[128,20] | -cq[128,1]
    cbase = np.zeros((128, 22), np.float32)
    for p in range(128):
        cbase[p, 0] = c[min(p // O, G - 1)]
        cbase[p, 21] = -c[min(p // O, G - 1)]
    for p in range(G * O):
        cbase[p, 1 + p % O] = kappa          # num mask
        cbase[p, 11 + p % O] = kappa         # den mask
    return cbase


def _build(h: float):
    import concourse.bass as bass
    import concourse.bacc as bacc
    import concourse.tile as tile
    from concourse import mybir

    f32 = mybir.dt.float32
    bf16 = mybir.dt.bfloat16
    AF = mybir.ActivationFunctionType
    ALU = mybir.AluOpType

    s_n = 1.0 / (h * h)

    nc = bacc.Bacc("TRN2", target_bir_lowering=False, debug=False, num_devices=1)
    tXd = [nc.dram_tensor(f"tX{q}", [128, 512], bf16, kind="ExternalInput").ap()
           for q in range(NPAIR)]
    Yrd = [nc.dram_tensor(f"Yr{q}", [128, 1024], bf16, kind="ExternalInput").ap()
           for q in range(NPAIR)]
    wpd = nc.dram_tensor("wpack", [128, 256], bf16, kind="ExternalInput").ap()
    cpd = nc.dram_tensor("cpack", [128, 62], f32, kind="ExternalInput").ap()
    out = nc.dram_tensor("out", [BQ, O], f32, kind="ExternalOutput").ap()

    with tile.TileContext(nc) as tc:
        with (
            tc.tile_pool(name="S", bufs=1) as S,
            tc.tile_pool(name="W", bufs=2) as W,
            tc.tile_pool(name="PS", bufs=1, space="PSUM") as PS,
        ):
            # ---- ACT warmup (loads the multi-func table once) ----
            warm = S.tile([1, 16], f32)
            nc.vector.memset(warm, 0.0)
            nc.scalar.activation(out=warm, in_=warm, func=AF.Exp)

            # ---- input DMAs: spread across engine queues, big+contiguous
            wp = S.tile([128, 256], bf16)
            nc.sync.dma_start(out=wp, in_=wpd)
            cp = S.tile([128, 62], f32)
            nc.scalar.dma_start(out=cp, in_=cpd)
            tX = [S.tile([128, 512], bf16, name=f"tX{q}") for q in range(NPAIR)]
            Yr = [S.tile([128, 1024], bf16, name=f"Yr{q}") for q in range(NPAIR)]
            nc.sync.dma_start(out=tX[0], in_=tXd[0])
            nc.scalar.dma_start(out=tX[1], in_=tXd[1])
            nc.gpsimd.dma_start(out=tX[2], in_=tXd[2])
            nc.gpsimd.dma_start(out=tX[3], in_=tXd[3])
            nc.sync.dma_start(out=Yr[0], in_=Yrd[0])
            nc.scalar.dma_start(out=Yr[1], in_=Yrd[1])
            nc.sync.dma_start(out=Yr[2], in_=Yrd[2])
            nc.gpsimd.dma_start(out=Yr[3], in_=Yrd[3])

            w1a = wp[0:64, 0:128]
            w1b = wp[64:128, 0:128]
            w2r = wp[:, 128:256]
            cq = cp[:, 0:1]
            kmask2 = cp[:, 1:21]
            ncq = cp[:, 21:22]
            ydt = cp[:, 22:62]

            nparts = S.tile([128, NPAIR], f32)
            dparts = S.tile([128, NPAIR], f32)
            parts = S.tile([128, 2], f32)
            AA = S.tile([128, 2 * O], bf16)
            ET0 = S.tile([128, 1024], bf16)
            nsb = S.tile([128, 4 * O], f32)
            dsb = S.tile([128, 4 * O], f32)
            rsb = S.tile([128, 4 * O], f32)
            osb = S.tile([128, 4 * O], f32)

            # ---- 4 train pairs, software-pipelined ----
            hps_t = []

            def w1_pair(q):
                hps = PS.tile([128, 1024], f32, tag="hps", bufs=2,
                              name=f"hps{q}")
                hps_t.append(hps)
                nc.tensor.matmul(hps[:, 0:512], lhsT=w1a, rhs=tX[q][0:64, :],
                                 start=True, stop=True)
                nc.tensor.matmul(hps[:, 512:1024], lhsT=w1b,
                                 rhs=tX[q][64:128, :],
                                 start=True, stop=True)

            w1_pair(0)
            for q in range(NPAIR):
                h1 = W.tile([128, 1024], bf16, tag="h1")
                if q % 2 == 0:
                    # DVE relu (PSUM read), ACT does square this pair
                    nc.vector.tensor_scalar(out=h1, in0=hps_t[q], scalar1=0.0,
                                            scalar2=None, op0=ALU.max)
                else:
                    nc.scalar.activation(out=h1, in_=hps_t[q], func=AF.Relu)
                if q + 1 < NPAIR:
                    w1_pair(q + 1)
                xr = PS.tile([128, 1024], f32, tag="xr", bufs=2, name=f"xr{q}")
                nc.tensor.matmul(xr[:, 0:512], lhsT=w2r, rhs=h1[:, 0:512],
                                 start=True, stop=True)
                nc.tensor.matmul(xr[:, 512:1024], lhsT=w2r, rhs=h1[:, 512:1024],
                                 start=True, stop=True)
                d2 = W.tile([128, 1024], f32, tag="d2")
                if q % 2 == 0:
                    nc.scalar.activation(out=d2, in_=xr, func=AF.Square,
                                         bias=ncq, scale=1.0)
                else:
                    db = W.tile([128, 1024], bf16, tag="db")
                    nc.vector.tensor_scalar(out=db, in0=xr, scalar1=cq,
                                            scalar2=None, op0=ALU.subtract)
                    nc.gpsimd.tensor_tensor(out=d2, in0=db, in1=db, op=ALU.mult)
                ET = ET0 if q == 0 else W.tile([128, 1024], bf16, tag="ET")
                nc.scalar.activation(out=ET, in_=d2, func=AF.Exp, scale=-s_n,
                                     accum_out=dparts[:, q:q + 1])
                scr = W.tile([128, 1024], bf16, tag="scr")
                nc.vector.scalar_tensor_tensor(
                    out=scr, in0=ET, scalar=1.0, in1=Yr[q],
                    op0=ALU.bypass, op1=ALU.mult,
                    accum_out=nparts[:, q:q + 1])

            # ---- tables -> AA (bf16 for the bf16 fin matmuls) ----
            nc.vector.tensor_reduce(out=parts[:, 0:1], in_=nparts,
                                    axis=mybir.AxisListType.X, op=ALU.add)
            nc.vector.tensor_reduce(out=parts[:, 1:2], in_=dparts,
                                    axis=mybir.AxisListType.X, op=ALU.add)
            PP = parts.ap[0][0]
            parts_b = bass.AP(tensor=parts.tensor, offset=parts.offset,
                              ap=[[PP, 128], [1, 2], [0, O]])
            nc.vector.tensor_tensor(out=AA.rearrange("p (k e) -> p k e", e=O),
                                    in0=parts_b,
                                    in1=kmask2.rearrange("p (k e) -> p k e", e=O),
                                    op=ALU.mult)

            # ---- query contraction directly into query-partition layout
            fin = PS.tile([128, 4 * 2 * O], f32, tag="xr", bufs=2, name="fin")
            for j in range(4):
                nc.tensor.matmul(fin[:, j * 20:(j + 1) * 20],
                                 lhsT=ET0[:, j * 128:(j + 1) * 128], rhs=AA,
                                 start=True, stop=True)

            FP = fin.ap[0][0]
            num4 = bass.AP(tensor=fin.tensor, offset=fin.offset,
                           ap=[[FP, 128], [2 * O, 4], [1, O]])
            den4 = bass.AP(tensor=fin.tensor, offset=fin.offset + O,
                           ap=[[FP, 128], [2 * O, 4], [1, O]])
            nc.vector.tensor_tensor(out=nsb, in0=num4, in1=ydt, op=ALU.subtract)
            nc.vector.tensor_scalar(out=dsb, in0=den4, scalar1=-1.0,
                                    scalar2=None, op0=ALU.add)
            nc.vector.reciprocal(rsb, dsb)
            nc.vector.tensor_tensor(out=osb, in0=nsb, in1=rsb, op=ALU.mult)
            nc.sync.dma_start(
                out=out.rearrange("(j p) o -> p j o", p=128),
                in_=osb.rearrange("p (j o) -> p j o", o=O))

    nc.compile()
    return nc


def build_in_maps(x, train_X, Y, W1, W2, h):
    import jax.numpy as jnp

    def bf(a):
        return np.asarray(jnp.asarray(a, dtype=jnp.bfloat16))

    cbase = _host_consts(float(h))
    x = np.ascontiguousarray(x, dtype=np.float32)
    train_X = np.ascontiguousarray(train_X, dtype=np.float32)
    Y = np.ascontiguousarray(Y, dtype=np.float32)
    W1 = np.ascontiguousarray(W1, dtype=np.float32)
    W2 = np.ascontiguousarray(W2, dtype=np.float32)

    pmod = np.arange(128) % O
    wpack = np.zeros((128, 256), np.float32)
    wpack[0:64, 0:128] = W1.T
    wpack[64:128, 0:128] = W1.T
    wpack[:, 128:256] = W2[pmod].T      # W2rep[k, p] = W2[p%10, k]
    wpack16 = bf(wpack)

    idx = np.arange(N)
    in_maps = []
    for c in range(NCORES):
        n_list = (idx + c * BQ) % N     # core's own queries first
        Xp = train_X[n_list]            # [N, 64]
        Yp = Y[n_list][:, pmod]         # [N, 128]
        m = {"wpack": wpack16}
        for q in range(NPAIR):
            h0 = slice(q * 512, (q + 1) * 512)
            h1 = slice(2048 + q * 512, 2048 + (q + 1) * 512)
            m[f"tX{q}"] = bf(np.concatenate([Xp[h0].T, Xp[h1].T], axis=0))
            m[f"Yr{q}"] = bf(np.concatenate([Yp[h0].T, Yp[h1].T], axis=1))
        cpack = np.zeros((128, 62), np.float32)
        cpack[:, 0:22] = cbase
        ydt = Y[c * BQ:(c + 1) * BQ].reshape(4, 128, O).transpose(1, 0, 2)
        cpack[:, 22:62] = ydt.reshape(128, 4 * O)
        m["cpack"] = cpack
        in_maps.append(m)
    return in_maps


def kernel(x, train_X, Y, W1, W2, h):
    import concourse.bass_utils as bass_utils

    hval = float(h)
    key = ("fgt3", hval)
    if key not in _cache:
        _cache[key] = _build(hval)
    nc = _cache[key]

    in_maps = build_in_maps(x, train_X, Y, W1, W2, h)
    res = bass_utils.run_bass_kernel_spmd(nc, in_maps, core_ids=list(range(NCORES)))
    return np.concatenate([res.results[c]["out"] for c in range(NCORES)], axis=0)


# revision 7
# speedup vs baseline: 1.3154x; 1.0284x over previous
"""Trainium2 Bass kernel for leave-one-out Nadaraya-Watson regression
(nn_Net_72877005078649) — fast-Gauss-transform formulation, v3.

Per output channel o this is 1D Gaussian kernel regression; the kernel
factorizes through a G=12 grid (a = b = h/sqrt(2), trapezoid aliasing
~1e-4):  K_h(x,z) ~= kappa * sum_g exp(-(c_g-x)^2/h^2) exp(-(z-c_g)^2/h^2)

v3 design notes (vs v2 baseline at ~41.5us):
 - per-core input roll: core c's train data is rotated so its own 512
   queries are train chunk 0 — the query-side Eq is just cols 0:512 of
   pair-0's train-side exp table ET0. The whole separate query
   projection chain (W1/relu/W2/sub/sq/exp on [*,512]) is gone.
 - inputs ship packed on 128 partitions, pair-contiguous: tX_q
   [128,512] bf16 holds two 512-col train chunks stacked on partition
   halves (W1T duplicated on rows 0:64 and 64:128 of wpack so both
   halves matmul with base-partition-aligned lhsT). Few large
   contiguous DMAs across 4 engine queues instead of many tiny-packet
   strided ones.
 - finalize: fin_j [128q,20] = matmul(lhsT=ET0[:, j*128:+128] bf16,
   rhs=AA bf16) gives num|den directly in query-partition layout — no
   identity transposes, no PSUM->SBUF copy, no make_identity.
 - elementwise rebalance per pair across ACT/DVE/GpSimd; ACT exp keeps
   den via accum_out, num via STT (DVE on odd / GpSimd on even pairs).
 - diagonal: train_X == x by construction, K_ii == 1 exactly:
   out = (num - Y_d)/(den - 1).

Sharding: queries split across 8 cores (512/core); train replicated.
"""

import numpy as np

N = 4096
D = 64
HID = 128
O = 10
NCORES = 8
BQ = N // NCORES
G = 12
GRID_LO = -6.5
GRID_HI = 6.5
NPAIR = 4               # train chunk pairs, 1024 cols each

_cache = {}


def _host_consts(h: float):
    c = np.linspace(GRID_LO, GRID_HI, G).astype(np.float32)
    delta = float(c[1] - c[0])
    kappa = 2.0 * delta / (np.sqrt(2.0 * np.pi) * h)
    # cbase[128, 22] = cq[128,1] | kmask2[128,20] | -cq[128,1]
    cbase = np.zeros((128, 22), np.float32)
    for p in range(128):
        cbase[p, 0] = c[min(p // O, G - 1)]
        cbase[p, 21] = -c[min(p // O, G - 1)]
    for p in range(G * O):
        cbase[p, 1 + p % O] = kappa          # num mask
        cbase[p, 11 + p % O] = kappa         # den mask
    return cbase


def _build(h: float):
    import concourse.bass as bass
    import concourse.bacc as bacc
    import concourse.tile as tile
    from concourse import mybir

    f32 = mybir.dt.float32
    bf16 = mybir.dt.bfloat16
    AF = mybir.ActivationFunctionType
    ALU = mybir.AluOpType

    s_n = 1.0 / (h * h)

    nc = bacc.Bacc("TRN2", target_bir_lowering=False, debug=False, num_devices=1)
    tXd = [nc.dram_tensor(f"tX{q}", [128, 512], bf16, kind="ExternalInput").ap()
           for q in range(NPAIR)]
    Yrd = [nc.dram_tensor(f"Yr{q}", [128, 1024], bf16, kind="ExternalInput").ap()
           for q in range(NPAIR)]
    wpd = nc.dram_tensor("wpack", [128, 256], bf16, kind="ExternalInput").ap()
    cpd = nc.dram_tensor("cpack", [128, 62], f32, kind="ExternalInput").ap()
    out = nc.dram_tensor("out", [BQ, O], f32, kind="ExternalOutput").ap()

    with tile.TileContext(nc) as tc:
        with (
            tc.tile_pool(name="S", bufs=1) as S,
            tc.tile_pool(name="W", bufs=2) as W,
            tc.tile_pool(name="PS", bufs=1, space="PSUM") as PS,
        ):
            # ---- ACT warmup (loads the multi-func table once) ----
            warm = S.tile([1, 16], f32)
            nc.vector.memset(warm, 0.0)
            nc.scalar.activation(out=warm, in_=warm, func=AF.Exp)

            # ---- input DMAs: spread across engine queues, big+contiguous
            wp = S.tile([128, 256], bf16)
            cp = S.tile([128, 62], f32)
            tX = [S.tile([128, 512], bf16, name=f"tX{q}") for q in range(NPAIR)]
            Yr = [S.tile([128, 1024], bf16, name=f"Yr{q}") for q in range(NPAIR)]
            # priority order: weights + first train chunks first, Y tables last
            nc.sync.dma_start(out=tX[0], in_=tXd[0])
            nc.scalar.dma_start(out=wp, in_=wpd)
            nc.gpsimd.dma_start(out=tX[2], in_=tXd[2])
            nc.sync.dma_start(out=tX[1], in_=tXd[1])
            nc.scalar.dma_start(out=cp, in_=cpd)
            nc.gpsimd.dma_start(out=tX[3], in_=tXd[3])
            nc.sync.dma_start(out=Yr[0], in_=Yrd[0])
            nc.scalar.dma_start(out=Yr[1], in_=Yrd[1])
            nc.sync.dma_start(out=Yr[2], in_=Yrd[2])
            nc.gpsimd.dma_start(out=Yr[3], in_=Yrd[3])

            w1a = wp[0:64, 0:128]
            w1b = wp[64:128, 0:128]
            w2r = wp[:, 128:256]
            cq = cp[:, 0:1]
            kmask2 = cp[:, 1:21]
            ncq = cp[:, 21:22]
            ydt = cp[:, 22:62]

            nparts = S.tile([128, NPAIR + 1], f32)
            dparts = S.tile([128, NPAIR + 1], f32)
            parts = S.tile([128, 2], f32)
            AA = S.tile([128, 2 * O], bf16)
            ET0 = S.tile([128, 1024], bf16)
            nsb = S.tile([128, 4 * O], f32)
            dsb = S.tile([128, 4 * O], f32)
            rsb = S.tile([128, 4 * O], f32)
            osb = S.tile([128, 4 * O], f32)

            # ---- 4 train pairs, software-pipelined ----
            hps_t = []

            def w1_pair(q):
                hps = PS.tile([128, 1024], f32, tag="hps", bufs=2,
                              name=f"hps{q}")
                hps_t.append(hps)
                nc.tensor.matmul(hps[:, 0:512], lhsT=w1a, rhs=tX[q][0:64, :],
                                 start=True, stop=True)
                nc.tensor.matmul(hps[:, 512:1024], lhsT=w1b,
                                 rhs=tX[q][64:128, :],
                                 start=True, stop=True)

            w1_pair(0)
            for q in range(NPAIR):
                last = q == NPAIR - 1
                h1 = W.tile([128, 1024], bf16, tag="h1", bufs=3)
                if last:
                    # split halves across engines: shorter serial drain
                    nc.vector.tensor_scalar(out=h1[:, 0:512],
                                            in0=hps_t[q][:, 0:512],
                                            scalar1=0.0, scalar2=None,
                                            op0=ALU.max)
                    nc.scalar.activation(out=h1[:, 512:1024],
                                         in_=hps_t[q][:, 512:1024],
                                         func=AF.Relu)
                else:
                    nc.vector.tensor_scalar(out=h1, in0=hps_t[q], scalar1=0.0,
                                            scalar2=None, op0=ALU.max)
                if q + 1 < NPAIR:
                    w1_pair(q + 1)
                xr = PS.tile([128, 1024], f32, tag="xr", bufs=2, name=f"xr{q}")
                nc.tensor.matmul(xr[:, 0:512], lhsT=w2r, rhs=h1[:, 0:512],
                                 start=True, stop=True)
                nc.tensor.matmul(xr[:, 512:1024], lhsT=w2r, rhs=h1[:, 512:1024],
                                 start=True, stop=True)
                d2 = W.tile([128, 1024], f32, tag="d2", bufs=3)
                ET = ET0 if q == 0 else W.tile([128, 1024], bf16, tag="ET",
                                               bufs=3)
                scr = W.tile([128, 1024], bf16, tag="scr", bufs=3)
                if last:
                    for hh in range(2):
                        sl = slice(hh * 512, (hh + 1) * 512)
                        nc.scalar.activation(out=d2[:, sl], in_=xr[:, sl],
                                             func=AF.Square, bias=ncq,
                                             scale=1.0)
                        nc.scalar.activation(out=ET[:, sl], in_=d2[:, sl],
                                             func=AF.Exp, scale=-s_n,
                                             accum_out=dparts[:, q + hh:q + hh + 1])
                        nc.vector.scalar_tensor_tensor(
                            out=scr[:, sl], in0=ET[:, sl], scalar=1.0,
                            in1=Yr[q][:, sl],
                            op0=ALU.bypass, op1=ALU.mult,
                            accum_out=nparts[:, q + hh:q + hh + 1])
                else:
                    nc.scalar.activation(out=d2, in_=xr, func=AF.Square,
                                         bias=ncq, scale=1.0)
                    nc.scalar.activation(out=ET, in_=d2, func=AF.Exp,
                                         scale=-s_n,
                                         accum_out=dparts[:, q:q + 1])
                    nc.vector.scalar_tensor_tensor(
                        out=scr, in0=ET, scalar=1.0, in1=Yr[q],
                        op0=ALU.bypass, op1=ALU.mult,
                        accum_out=nparts[:, q:q + 1])

            # ---- tables -> AA (bf16 for the bf16 fin matmuls) ----
            nc.vector.tensor_reduce(out=parts[:, 0:1], in_=nparts,
                                    axis=mybir.AxisListType.X, op=ALU.add)
            nc.vector.tensor_reduce(out=parts[:, 1:2], in_=dparts,
                                    axis=mybir.AxisListType.X, op=ALU.add)
            PP = parts.ap[0][0]
            parts_b = bass.AP(tensor=parts.tensor, offset=parts.offset,
                              ap=[[PP, 128], [1, 2], [0, O]])
            nc.vector.tensor_tensor(out=AA.rearrange("p (k e) -> p k e", e=O),
                                    in0=parts_b,
                                    in1=kmask2.rearrange("p (k e) -> p k e", e=O),
                                    op=ALU.mult)

            # ---- query contraction directly into query-partition layout
            fin = PS.tile([128, 4 * 2 * O], f32, tag="xr", bufs=2, name="fin")
            for j in range(4):
                nc.tensor.matmul(fin[:, j * 20:(j + 1) * 20],
                                 lhsT=ET0[:, j * 128:(j + 1) * 128], rhs=AA,
                                 start=True, stop=True)

            FP = fin.ap[0][0]
            num4 = bass.AP(tensor=fin.tensor, offset=fin.offset,
                           ap=[[FP, 128], [2 * O, 4], [1, O]])
            den4 = bass.AP(tensor=fin.tensor, offset=fin.offset + O,
                           ap=[[FP, 128], [2 * O, 4], [1, O]])
            nc.vector.tensor_tensor(out=nsb, in0=num4, in1=ydt, op=ALU.subtract)
            nc.vector.tensor_scalar(out=dsb, in0=den4, scalar1=-1.0,
                                    scalar2=None, op0=ALU.add)
            nc.vector.reciprocal(rsb, dsb)
            nc.vector.tensor_tensor(out=osb, in0=nsb, in1=rsb, op=ALU.mult)
            nc.sync.dma_start(
                out=out.rearrange("(j p) o -> p j o", p=128),
                in_=osb.rearrange("p (j o) -> p j o", o=O))

    nc.compile()
    return nc


def build_in_maps(x, train_X, Y, W1, W2, h):
    import jax.numpy as jnp

    def bf(a):
        return np.asarray(jnp.asarray(a, dtype=jnp.bfloat16))

    cbase = _host_consts(float(h))
    x = np.ascontiguousarray(x, dtype=np.float32)
    train_X = np.ascontiguousarray(train_X, dtype=np.float32)
    Y = np.ascontiguousarray(Y, dtype=np.float32)
    W1 = np.ascontiguousarray(W1, dtype=np.float32)
    W2 = np.ascontiguousarray(W2, dtype=np.float32)

    pmod = np.arange(128) % O
    wpack = np.zeros((128, 256), np.float32)
    wpack[0:64, 0:128] = W1.T
    wpack[64:128, 0:128] = W1.T
    wpack[:, 128:256] = W2[pmod].T      # W2rep[k, p] = W2[p%10, k]
    wpack16 = bf(wpack)

    idx = np.arange(N)
    in_maps = []
    for c in range(NCORES):
        n_list = (idx + c * BQ) % N     # core's own queries first
        Xp = train_X[n_list]            # [N, 64]
        Yp = Y[n_list][:, pmod]         # [N, 128]
        m = {"wpack": wpack16}
        for q in range(NPAIR):
            h0 = slice(q * 512, (q + 1) * 512)
            h1 = slice(2048 + q * 512, 2048 + (q + 1) * 512)
            m[f"tX{q}"] = bf(np.concatenate([Xp[h0].T, Xp[h1].T], axis=0))
            m[f"Yr{q}"] = bf(np.concatenate([Yp[h0].T, Yp[h1].T], axis=1))
        cpack = np.zeros((128, 62), np.float32)
        cpack[:, 0:22] = cbase
        ydt = Y[c * BQ:(c + 1) * BQ].reshape(4, 128, O).transpose(1, 0, 2)
        cpack[:, 22:62] = ydt.reshape(128, 4 * O)
        m["cpack"] = cpack
        in_maps.append(m)
    return in_maps


def kernel(x, train_X, Y, W1, W2, h):
    import concourse.bass_utils as bass_utils

    hval = float(h)
    key = ("fgt3", hval)
    if key not in _cache:
        _cache[key] = _build(hval)
    nc = _cache[key]

    in_maps = build_in_maps(x, train_X, Y, W1, W2, h)
    res = bass_utils.run_bass_kernel_spmd(nc, in_maps, core_ids=list(range(NCORES)))
    return np.concatenate([res.results[c]["out"] for c in range(NCORES)], axis=0)


# revision 12
# speedup vs baseline: 1.3188x; 1.0026x over previous
"""Trainium2 Bass kernel for leave-one-out Nadaraya-Watson regression
(nn_Net_72877005078649) — fast-Gauss-transform formulation, v3.

Per output channel o this is 1D Gaussian kernel regression; the kernel
factorizes through a G=12 grid (a = b = h/sqrt(2), trapezoid aliasing
~1e-4):  K_h(x,z) ~= kappa * sum_g exp(-(c_g-x)^2/h^2) exp(-(z-c_g)^2/h^2)

v3 design notes (vs v2 baseline at ~41.5us):
 - per-core input roll: core c's train data is rotated so its own 512
   queries are train chunk 0 — the query-side Eq is just cols 0:512 of
   pair-0's train-side exp table ET0. The whole separate query
   projection chain (W1/relu/W2/sub/sq/exp on [*,512]) is gone.
 - inputs ship packed on 128 partitions, pair-contiguous: tX_q
   [128,512] bf16 holds two 512-col train chunks stacked on partition
   halves (W1T duplicated on rows 0:64 and 64:128 of wpack so both
   halves matmul with base-partition-aligned lhsT). Few large
   contiguous DMAs across 4 engine queues instead of many tiny-packet
   strided ones.
 - finalize: fin_j [128q,20] = matmul(lhsT=ET0[:, j*128:+128] bf16,
   rhs=AA bf16) gives num|den directly in query-partition layout — no
   identity transposes, no PSUM->SBUF copy, no make_identity.
 - elementwise rebalance per pair across ACT/DVE/GpSimd; ACT exp keeps
   den via accum_out, num via STT (DVE on odd / GpSimd on even pairs).
 - diagonal: train_X == x by construction, K_ii == 1 exactly:
   out = (num - Y_d)/(den - 1).

Sharding: queries split across 8 cores (512/core); train replicated.
"""

import numpy as np

N = 4096
D = 64
HID = 128
O = 10
NCORES = 8
BQ = N // NCORES
G = 12
GRID_LO = -6.5
GRID_HI = 6.5
NPAIR = 4               # train chunk pairs, 1024 cols each

_cache = {}


def _host_consts(h: float):
    c = np.linspace(GRID_LO, GRID_HI, G).astype(np.float32)
    delta = float(c[1] - c[0])
    kappa = 2.0 * delta / (np.sqrt(2.0 * np.pi) * h)
    # cbase[128, 22] = cq[128,1] | kmask2[128,20] | -cq[128,1]
    # ET is computed via Derivative_Erf = (2/sqrt(pi))*exp(-u^2); both the
    # query and train factors carry 2/sqrt(pi), so fold (pi/4) into kappa.
    kap = kappa * np.pi / 4.0
    cbase = np.zeros((128, 22), np.float32)
    for p in range(128):
        cbase[p, 0] = c[min(p // O, G - 1)]
        cbase[p, 21] = -c[min(p // O, G - 1)] / h   # DErf bias: -c_g/h
    for p in range(G * O):
        cbase[p, 1 + p % O] = kap            # num mask
        cbase[p, 11 + p % O] = kap           # den mask
    return cbase


def _build(h: float):
    import concourse.bass as bass
    import concourse.bacc as bacc
    import concourse.tile as tile
    from concourse import mybir

    f32 = mybir.dt.float32
    bf16 = mybir.dt.bfloat16
    AF = mybir.ActivationFunctionType
    ALU = mybir.AluOpType

    inv_h = 1.0 / h

    nc = bacc.Bacc("TRN2", target_bir_lowering=False, debug=False, num_devices=1)
    tXd = [nc.dram_tensor(f"tX{q}", [128, 512], bf16, kind="ExternalInput").ap()
           for q in range(NPAIR)]
    Yrd = [nc.dram_tensor(f"Yr{q}", [128, 1024], bf16, kind="ExternalInput").ap()
           for q in range(NPAIR)]
    wpd = nc.dram_tensor("wpack", [128, 256], bf16, kind="ExternalInput").ap()
    cpd = nc.dram_tensor("cpack", [128, 62], f32, kind="ExternalInput").ap()
    out = nc.dram_tensor("out", [BQ, O], f32, kind="ExternalOutput").ap()

    with tile.TileContext(nc) as tc:
        with (
            tc.tile_pool(name="S", bufs=1) as S,
            tc.tile_pool(name="W", bufs=2) as W,
            tc.tile_pool(name="PS", bufs=1, space="PSUM") as PS,
        ):
            # ---- ACT warmup (loads the multi-func table once) ----
            warm = S.tile([1, 16], f32)
            nc.vector.memset(warm, 0.0)
            nc.scalar.activation(out=warm, in_=warm, func=AF.Derivative_Erf)

            # ---- input DMAs: spread across engine queues, big+contiguous
            wp = S.tile([128, 256], bf16)
            cp = S.tile([128, 62], f32)
            tX = [S.tile([128, 512], bf16, name=f"tX{q}") for q in range(NPAIR)]
            Yr = [S.tile([128, 1024], bf16, name=f"Yr{q}") for q in range(NPAIR)]
            # priority order: weights + first train chunks first, Y tables last
            nc.sync.dma_start(out=tX[0], in_=tXd[0])
            nc.scalar.dma_start(out=wp, in_=wpd)
            nc.gpsimd.dma_start(out=tX[2], in_=tXd[2])
            nc.scalar.dma_start(out=cp, in_=cpd)
            nc.scalar.dma_start(out=tX[1], in_=tXd[1])
            nc.gpsimd.dma_start(out=tX[3], in_=tXd[3])
            nc.sync.dma_start(out=Yr[0], in_=Yrd[0])
            nc.scalar.dma_start(out=Yr[1], in_=Yrd[1])
            nc.sync.dma_start(out=Yr[2], in_=Yrd[2])
            nc.gpsimd.dma_start(out=Yr[3], in_=Yrd[3])

            w1a = wp[0:64, 0:128]
            w1b = wp[64:128, 0:128]
            w2r = wp[:, 128:256]
            cq = cp[:, 0:1]
            kmask2 = cp[:, 1:21]
            ncq = cp[:, 21:22]
            ydt = cp[:, 22:62]

            nparts = S.tile([128, NPAIR + 1], f32)
            dparts = S.tile([128, NPAIR + 1], f32)
            parts = S.tile([128, 2], f32)
            AA = S.tile([128, 2 * O], bf16)
            ET0 = S.tile([128, 1024], bf16)
            nsb = S.tile([128, 4 * O], f32)
            dsb = S.tile([128, 4 * O], f32)
            rsb = S.tile([128, 4 * O], f32)
            osb = S.tile([128, 4 * O], f32)

            # ---- 4 train pairs, software-pipelined ----
            hps_t = []

            def w1_pair(q):
                hps = PS.tile([128, 1024], f32, tag="hps", bufs=2,
                              name=f"hps{q}")
                hps_t.append(hps)
                nc.tensor.matmul(hps[:, 0:512], lhsT=w1a, rhs=tX[q][0:64, :],
                                 start=True, stop=True)
                nc.tensor.matmul(hps[:, 512:1024], lhsT=w1b,
                                 rhs=tX[q][64:128, :],
                                 start=True, stop=True)

            w1_pair(0)
            for q in range(NPAIR):
                last = q == NPAIR - 1
                h1 = W.tile([128, 1024], bf16, tag="h1", bufs=3)
                if last:
                    # split halves: shorter serial drain into the finalize
                    nc.vector.tensor_scalar(out=h1[:, 0:512],
                                            in0=hps_t[q][:, 0:512],
                                            scalar1=0.0, scalar2=None,
                                            op0=ALU.max)
                    nc.vector.tensor_scalar(out=h1[:, 512:1024],
                                            in0=hps_t[q][:, 512:1024],
                                            scalar1=0.0, scalar2=None,
                                            op0=ALU.max)
                elif q == 1:
                    # one relu on ACT to balance engine load
                    nc.scalar.activation(out=h1, in_=hps_t[q], func=AF.Relu)
                else:
                    nc.vector.tensor_scalar(out=h1, in0=hps_t[q], scalar1=0.0,
                                            scalar2=None, op0=ALU.max)
                if q + 1 < NPAIR:
                    w1_pair(q + 1)
                xr = PS.tile([128, 1024], f32, tag="xr", bufs=2, name=f"xr{q}")
                nc.tensor.matmul(xr[:, 0:512], lhsT=w2r, rhs=h1[:, 0:512],
                                 start=True, stop=True)
                nc.tensor.matmul(xr[:, 512:1024], lhsT=w2r, rhs=h1[:, 512:1024],
                                 start=True, stop=True)
                # fused Gaussian: DErf(xr/h - c_g/h) = 2/sqrt(pi) exp(-s(xr-c)^2)
                ET = ET0 if q == 0 else W.tile([128, 1024], bf16, tag="ET",
                                               bufs=3)
                scr = W.tile([128, 1024], bf16, tag="scr", bufs=3)
                if last:
                    for hh in range(2):
                        sl = slice(hh * 512, (hh + 1) * 512)
                        nc.scalar.activation(out=ET[:, sl], in_=xr[:, sl],
                                             func=AF.Derivative_Erf,
                                             bias=ncq, scale=inv_h,
                                             accum_out=dparts[:, q + hh:q + hh + 1])
                        nc.vector.scalar_tensor_tensor(
                            out=scr[:, sl], in0=ET[:, sl], scalar=1.0,
                            in1=Yr[q][:, sl],
                            op0=ALU.bypass, op1=ALU.mult,
                            accum_out=nparts[:, q + hh:q + hh + 1])
                else:
                    nc.scalar.activation(out=ET, in_=xr,
                                         func=AF.Derivative_Erf,
                                         bias=ncq, scale=inv_h,
                                         accum_out=dparts[:, q:q + 1])
                    nc.vector.scalar_tensor_tensor(
                        out=scr, in0=ET, scalar=1.0, in1=Yr[q],
                        op0=ALU.bypass, op1=ALU.mult,
                        accum_out=nparts[:, q:q + 1])

            # ---- tables -> AA (bf16 for the bf16 fin matmuls) ----
            nc.vector.tensor_reduce(out=parts[:, 0:1], in_=nparts,
                                    axis=mybir.AxisListType.X, op=ALU.add)
            nc.vector.tensor_reduce(out=parts[:, 1:2], in_=dparts,
                                    axis=mybir.AxisListType.X, op=ALU.add)
            PP = parts.ap[0][0]
            parts_b = bass.AP(tensor=parts.tensor, offset=parts.offset,
                              ap=[[PP, 128], [1, 2], [0, O]])
            nc.vector.tensor_tensor(out=AA.rearrange("p (k e) -> p k e", e=O),
                                    in0=parts_b,
                                    in1=kmask2.rearrange("p (k e) -> p k e", e=O),
                                    op=ALU.mult)

            # ---- query contraction directly into query-partition layout
            fin = PS.tile([128, 4 * 2 * O], f32, tag="xr", bufs=2, name="fin")
            for j in range(4):
                nc.tensor.matmul(fin[:, j * 20:(j + 1) * 20],
                                 lhsT=ET0[:, j * 128:(j + 1) * 128], rhs=AA,
                                 start=True, stop=True)

            FP = fin.ap[0][0]
            num4 = bass.AP(tensor=fin.tensor, offset=fin.offset,
                           ap=[[FP, 128], [2 * O, 4], [1, O]])
            den4 = bass.AP(tensor=fin.tensor, offset=fin.offset + O,
                           ap=[[FP, 128], [2 * O, 4], [1, O]])
            nc.vector.tensor_tensor(out=nsb, in0=num4, in1=ydt, op=ALU.subtract)
            nc.vector.tensor_scalar(out=dsb, in0=den4, scalar1=-1.0,
                                    scalar2=None, op0=ALU.add)
            nc.vector.reciprocal(rsb, dsb)
            nc.vector.tensor_tensor(out=osb, in0=nsb, in1=rsb, op=ALU.mult)
            nc.sync.dma_start(
                out=out.rearrange("(j p) o -> p j o", p=128),
                in_=osb.rearrange("p (j o) -> p j o", o=O))

    nc.compile()
    return nc


def build_in_maps(x, train_X, Y, W1, W2, h):
    import jax.numpy as jnp

    def bf(a):
        return np.asarray(jnp.asarray(a, dtype=jnp.bfloat16))

    cbase = _host_consts(float(h))
    x = np.ascontiguousarray(x, dtype=np.float32)
    train_X = np.ascontiguousarray(train_X, dtype=np.float32)
    Y = np.ascontiguousarray(Y, dtype=np.float32)
    W1 = np.ascontiguousarray(W1, dtype=np.float32)
    W2 = np.ascontiguousarray(W2, dtype=np.float32)

    pmod = np.arange(128) % O
    wpack = np.zeros((128, 256), np.float32)
    wpack[0:64, 0:128] = W1.T
    wpack[64:128, 0:128] = W1.T
    wpack[:, 128:256] = W2[pmod].T      # W2rep[k, p] = W2[p%10, k]
    wpack16 = bf(wpack)

    idx = np.arange(N)
    in_maps = []
    for c in range(NCORES):
        n_list = (idx + c * BQ) % N     # core's own queries first
        Xp = train_X[n_list]            # [N, 64]
        Yp = Y[n_list][:, pmod]         # [N, 128]
        m = {"wpack": wpack16}
        for q in range(NPAIR):
            h0 = slice(q * 512, (q + 1) * 512)
            h1 = slice(2048 + q * 512, 2048 + (q + 1) * 512)
            m[f"tX{q}"] = bf(np.concatenate([Xp[h0].T, Xp[h1].T], axis=0))
            m[f"Yr{q}"] = bf(np.concatenate([Yp[h0].T, Yp[h1].T], axis=1))
        cpack = np.zeros((128, 62), np.float32)
        cpack[:, 0:22] = cbase
        ydt = Y[c * BQ:(c + 1) * BQ].reshape(4, 128, O).transpose(1, 0, 2)
        cpack[:, 22:62] = ydt.reshape(128, 4 * O)
        m["cpack"] = cpack
        in_maps.append(m)
    return in_maps


def kernel(x, train_X, Y, W1, W2, h):
    import concourse.bass_utils as bass_utils

    hval = float(h)
    key = ("fgt3", hval)
    if key not in _cache:
        _cache[key] = _build(hval)
    nc = _cache[key]

    in_maps = build_in_maps(x, train_X, Y, W1, W2, h)
    res = bass_utils.run_bass_kernel_spmd(nc, in_maps, core_ids=list(range(NCORES)))
    return np.concatenate([res.results[c]["out"] for c in range(NCORES)], axis=0)
